# Initial kernel scaffold
#
"""Multihead attention (B=4, S=2048, E=1024, H=16, D=64) on 8 Trainium2 cores.

Sharding: core c = (batch b = c//2, head-half hh = c%2). Each core computes one
batch's attention for 8 heads (512 of the 1024 projection columns), producing a
partial output (row-split Wo); the host sums the two partials per batch.

On-chip layout keeps everything transposed: qT/kT are [d, s], scores are
[sk, sq], the output is [e, s]. Softmax denominators come free from a ones
column appended to V (M=65 matmul); exp needs no max subtraction because
scores ~ N(0,1). Matmuls run in float32r (full PE rate for free dim >= 256).
"""
import sys

sys.path.insert(0, "/opt/trn_rl_repo")

import numpy as np

import concourse.bacc as bacc
import concourse.mybir as mybir
import concourse.tile as tile
from concourse.bass_utils import run_bass_kernel_spmd
from concourse.masks import make_identity

E = 1024
H = 16
D = 64
B = 4
S = 2048
HH = E // 2          # projection cols per core
N_CORES = 8
P = 128
NCH = 4              # s-chunks of 512
CH = 512
f32 = mybir.dt.float32
f32r = mybir.dt.float32r
AF = mybir.ActivationFunctionType

_cached = {}


def _build():
    nc = bacc.Bacc(None, target_bir_lowering=False)

    xq = nc.declare_dram_parameter("xq", [S, E], f32, isOutput=False)
    xk = nc.declare_dram_parameter("xk", [S, E], f32, isOutput=False)
    xv = nc.declare_dram_parameter("xv", [S, E], f32, isOutput=False)
    wq = nc.declare_dram_parameter("wq", [P, 8, HH], f32, isOutput=False)
    wk = nc.declare_dram_parameter("wk", [P, 8, HH], f32, isOutput=False)
    wv = nc.declare_dram_parameter("wv", [P, 8, HH], f32, isOutput=False)
    bq_col = nc.declare_dram_parameter("bq_col", [P, 4], f32, isOutput=False)
    bk_col = nc.declare_dram_parameter("bk_col", [P, 4], f32, isOutput=False)
    bv_row = nc.declare_dram_parameter("bv_row", [1, HH], f32, isOutput=False)
    wo = nc.declare_dram_parameter("wo", [P, 4, E], f32, isOutput=False)
    bo_col = nc.declare_dram_parameter("bo_col", [P, 8], f32, isOutput=False)
    yT = nc.declare_dram_parameter("yT", [E, S], f32, isOutput=True)

    with tile.TileContext(nc) as tc:
        const = tc.tile_pool(name="const", bufs=1)
        qkv = tc.tile_pool(name="qkv", bufs=1)
        oup = tc.tile_pool(name="oup", bufs=1)
        const.__enter__(); qkv.__enter__(); oup.__enter__()

        ident = const.tile([P, P], f32)
        make_identity(nc, ident[:])

        onesf = const.tile([P, P], f32)
        nc.vector.memset(onesf[:], 1.0)
        # f32r constants (memset can't target f32r; cast-copy from fp32)
        ones_t = const.tile([P, P], f32r)       # rows 64 and 96 used as bcast lhsT
        nc.vector.tensor_copy(ones_t[64:65, :], onesf[64:65, :])
        nc.vector.tensor_copy(ones_t[96:97, :], onesf[96:97, :])
        onesk1 = const.tile([1, P], f32r)       # lhsT for v-bias matmul
        nc.vector.tensor_copy(onesk1[:], onesf[0:1, :])
        vones = const.tile([P, 16, 8], f32)     # ones column filler for vbuf
        nc.vector.memset(vones[:], 1.0)

        bqc = const.tile([P, 4], f32)
        bkc = const.tile([P, 4], f32)
        boc = const.tile([P, 8], f32)
        bvr = const.tile([1, HH], f32r)
        nc.sync.dma_start(out=bqc[:], in_=bq_col[:])
        nc.sync.dma_start(out=bkc[:], in_=bk_col[:])
        nc.sync.dma_start(out=boc[:], in_=bo_col[:])
        nc.gpsimd.dma_start(out=bvr[:], in_=bv_row[:])

        qT = qkv.tile([P, 4, S], f32r)          # [dq within tile, pair, sq]
        kT = qkv.tile([P, 4, S], f32r)
        vbuf = qkv.tile([P, 16, 8, D + 1], f32r)  # [sv, s-tile, head, d|1]
        ou = oup.tile([P, 4, S], f32r)          # normalized attn out, [hd, pair, sq]

        nc.vector.tensor_copy(vbuf[:, :, :, D], vones[:])

        # ---------------- Phase A: transposes + projections ----------------
        with tc.tile_pool(name="wp", bufs=1) as wp, \
             tc.tile_pool(name="xp", bufs=3) as xp, \
             tc.tile_pool(name="xtp", bufs=1) as xtp, \
             tc.tile_pool(name="ps_tr", bufs=3, space="PSUM") as ps_tr, \
             tc.tile_pool(name="ps_pj", bufs=3, space="PSUM") as ps_pj:
            for xdram, wdram, kind in ((xv, wv, "v"), (xk, wk, "k"), (xq, wq, "q")):
                w_t = wp.tile([P, 8, HH], f32r, tag="w")
                nc.gpsimd.dma_start(out=w_t[:], in_=wdram[:])
                for c in range(NCH):
                    xT_t = xtp.tile([P, 8, CH], f32r, tag="xT")
                    for i in range(4):
                        x_t = xp.tile([P, E], f32, tag="x")
                        r0 = (c * 4 + i) * P
                        nc.sync.dma_start(out=x_t[:], in_=xdram[r0:r0 + P, :])
                        for et in range(8):
                            pt = ps_tr.tile([P, P], f32, tag="tr")
                            nc.tensor.transpose(pt[:], x_t[:, et * P:(et + 1) * P], ident[:])
                            nc.vector.tensor_copy(xT_t[:, et, i * P:(i + 1) * P], pt[:])
                    if kind == "v":
                        for i in range(4):
                            pp = ps_pj.tile([P, 8, D], f32, tag="pj")
                            for et in range(8):
                                nc.tensor.matmul(pp[:], lhsT=xT_t[:, et, i * P:(i + 1) * P],
                                                 rhs=w_t[:, et, :], start=(et == 0), stop=False)
                            nc.tensor.matmul(pp[:], lhsT=onesk1[:], rhs=bvr[:],
                                             start=False, stop=True)
                            nc.vector.tensor_copy(vbuf[:, c * 4 + i, :, 0:D], pp[:])
                    else:
                        bcol = bqc if kind == "q" else bkc
                        dest = qT if kind == "q" else kT
                        for dt in range(4):
                            pp = ps_pj.tile([P, CH], f32, tag="pj")
                            for et in range(8):
                                nc.tensor.matmul(pp[:], lhsT=w_t[:, et, dt * P:(dt + 1) * P],
                                                 rhs=xT_t[:, et, :], start=(et == 0), stop=(et == 7))
                            nc.scalar.activation(dest[:, dt, c * CH:(c + 1) * CH], pp[:],
                                                 AF.Identity, bias=bcol[:, dt:dt + 1], scale=1.0)

        # ---------------- Phase B: attention + softmax + normalize ----------------
        with tc.tile_pool(name="ep", bufs=4) as ep, \
             tc.tile_pool(name="dnp", bufs=2) as dnp, \
             tc.tile_pool(name="rdp", bufs=2) as rdp, \
             tc.tile_pool(name="bcp", bufs=2) as bcp, \
             tc.tile_pool(name="ps_sc", bufs=2, space="PSUM") as ps_sc, \
             tc.tile_pool(name="ps_ac", bufs=2, space="PSUM") as ps_ac, \
             tc.tile_pool(name="ps_bc", bufs=2, space="PSUM") as ps_bc:
            for pr in range(4):
                hA, hB = 2 * pr, 2 * pr + 1
                for c in range(NCH):
                    cs = slice(c * CH, (c + 1) * CH)
                    psoA = ps_ac.tile([D + 1, CH], f32, tag="acc")
                    psoB = ps_ac.tile([D + 1, CH], f32, tag="acc")
                    for s in range(0, 16, 2):
                        for half, (pso, hh_) in enumerate(((psoA, hA), (psoB, hB))):
                            pb = slice(64 * half, 64 * half + 64)
                            psc = ps_sc.tile([P, 2, CH], f32, tag="sc")
                            for j in range(2):
                                st = s + j
                                nc.tensor.matmul(psc[:, j, :],
                                                 lhsT=kT[pb, pr, st * P:(st + 1) * P],
                                                 rhs=qT[pb, pr, cs],
                                                 start=True, stop=True)
                            ex = ep.tile([P, 2, CH], f32r, tag="expT")
                            nc.scalar.activation(ex[:], psc[:], AF.Exp, scale=0.125)
                            for j in range(2):
                                st = s + j
                                nc.tensor.matmul(pso[:], lhsT=vbuf[:, st, hh_, :],
                                                 rhs=ex[:, j, :],
                                                 start=(st == 0), stop=(st == 15),
                                                 skip_group_check=True)
                    # softmax denominators sit at psum row 64 (ones column)
                    den = dnp.tile([P, CH], f32, tag="den")
                    nc.vector.tensor_copy(den[64:65, :], psoA[64:65, :])
                    nc.vector.tensor_copy(den[96:97, :], psoB[64:65, :])
                    rden = rdp.tile([P, CH], f32r, tag="rden")
                    with nc.allow_low_precision(reason="softmax reciprocal in f32r"):
                        nc.vector.reciprocal(rden[64:65, :], den[64:65, :])
                        nc.vector.reciprocal(rden[96:97, :], den[96:97, :])
                    psb = ps_bc.tile([P, CH], f32, tag="bc")
                    nc.tensor.matmul(psb[0:64, :], lhsT=ones_t[64:65, 0:64],
                                     rhs=rden[64:65, :], start=True, stop=True,
                                     tile_position=(64, 0))
                    nc.tensor.matmul(psb[64:128, :], lhsT=ones_t[96:97, 0:64],
                                     rhs=rden[96:97, :], start=True, stop=True,
                                     tile_position=(96, 64))
                    bcs = bcp.tile([P, CH], f32, tag="bcs")
                    nc.vector.tensor_copy(bcs[:], psb[:])
                    with nc.allow_low_precision(reason="normalized attn out in f32r"):
                        nc.vector.tensor_mul(ou[0:64, pr, cs], psoA[0:64, :], bcs[0:64, :])
                        nc.vector.tensor_mul(ou[64:128, pr, cs], psoB[0:64, :], bcs[64:128, :])

        # ---------------- Phase C: output projection ----------------
        with tc.tile_pool(name="wop", bufs=1) as wop, \
             tc.tile_pool(name="otp", bufs=2) as otp, \
             tc.tile_pool(name="ps_ou", bufs=4, space="PSUM") as ps_ou:
            wo_t = wop.tile([P, 4, E], f32r)
            nc.gpsimd.dma_start(out=wo_t[:], in_=wo[:])
            for et in range(8):
                out_t = otp.tile([P, S], f32, tag="out")
                for c in range(NCH):
                    po = ps_ou.tile([P, CH], f32, tag="po")
                    for t in range(4):
                        nc.tensor.matmul(po[:], lhsT=wo_t[:, t, et * P:(et + 1) * P],
                                         rhs=ou[:, t, c * CH:(c + 1) * CH],
                                         start=(t == 0), stop=(t == 3))
                    nc.scalar.activation(out_t[:, c * CH:(c + 1) * CH], po[:],
                                         AF.Identity, bias=boc[:, et:et + 1], scale=1.0)
                nc.sync.dma_start(out=yT[et * P:(et + 1) * P, :], in_=out_t[:])

        const.__exit__(None, None, None)
        qkv.__exit__(None, None, None)
        oup.__exit__(None, None, None)

    nc.finalize()
    return nc


def _get_nc():
    if "nc" not in _cached:
        _cached["nc"] = _build()
    return _cached["nc"]


def _in_maps(query, key, value, Wq, bq, Wk, bk, Wv, bv, Wo, bo):
    query = np.asarray(query, np.float32)
    key = np.asarray(key, np.float32)
    value = np.asarray(value, np.float32)
    maps = []
    for c in range(N_CORES):
        b, hh = divmod(c, 2)
        sl = slice(hh * HH, (hh + 1) * HH)

        def wcols(W, nt):
            # [E, HH] -> [P, nt, HH-ish] with row tiles on partitions
            Ws = np.asarray(W, np.float32)[:, sl]
            return np.ascontiguousarray(Ws.reshape(nt, P, HH).transpose(1, 0, 2))

        wo_s = np.asarray(Wo, np.float32)[sl, :]                      # [512, E]
        wo_r = np.ascontiguousarray(wo_s.reshape(4, P, E).transpose(1, 0, 2))
        bo_c = (np.asarray(bo, np.float32).reshape(8, P).T if hh == 0
                else np.zeros((P, 8), np.float32))
        maps.append({
            "xq": np.ascontiguousarray(query[b]),
            "xk": np.ascontiguousarray(key[b]),
            "xv": np.ascontiguousarray(value[b]),
            "wq": wcols(Wq, 8),
            "wk": wcols(Wk, 8),
            "wv": wcols(Wv, 8),
            "bq_col": np.ascontiguousarray(np.asarray(bq, np.float32)[sl].reshape(4, P).T),
            "bk_col": np.ascontiguousarray(np.asarray(bk, np.float32)[sl].reshape(4, P).T),
            "bv_row": np.asarray(bv, np.float32)[sl].reshape(1, HH),
            "wo": wo_r,
            "bo_col": np.ascontiguousarray(bo_c),
        })
    return maps


def _assemble(results):
    outs = [results[c]["yT"] for c in range(N_CORES)]
    return np.stack([(outs[2 * b] + outs[2 * b + 1]).T for b in range(B)]).astype(np.float32)


def kernel(**inputs):
    nc = _get_nc()
    maps = _in_maps(**inputs)
    r = run_bass_kernel_spmd(nc, maps, list(range(N_CORES)))
    return _assemble(r.results)


def kernel_traced(**inputs):
    """Like kernel() but with NTFF tracing; returns (output, exec_time_ns)."""
    nc = _get_nc()
    maps = _in_maps(**inputs)
    r = run_bass_kernel_spmd(nc, maps, list(range(N_CORES)), trace=True)
    return _assemble(r.results), r.exec_time_ns


# revision 5
# speedup vs baseline: 1.2393x; 1.2393x over previous
"""Multihead attention (B=4, S=2048, E=1024, H=16, D=64) on 8 Trainium2 cores.

Sharding: core c = (batch b = c//2, head-half hh = c%2). Each core computes one
batch's attention for 8 heads (512 of the 1024 projection columns), producing a
partial output (row-split Wo); the host sums the two partials per batch.

On-chip layout keeps everything transposed: qT/kT are [d, s], scores are
[sk, sq], the output is [e, s]. Softmax denominators come free from a ones
column appended to V (M=65 matmul); exp needs no max subtraction because
scores ~ N(0,1). Matmuls run in float32r (full PE rate for free dim >= 256).
"""
import sys

sys.path.insert(0, "/opt/trn_rl_repo")

import numpy as np

import concourse.bacc as bacc
import concourse.mybir as mybir
import concourse.tile as tile
from concourse.bass_utils import run_bass_kernel_spmd
from concourse.masks import make_identity

E = 1024
H = 16
D = 64
B = 4
S = 2048
HH = E // 2          # projection cols per core
N_CORES = 8
P = 128
NCH = 4              # s-chunks of 512
CH = 512
f32 = mybir.dt.float32
f32r = mybir.dt.float32r
AF = mybir.ActivationFunctionType

_cached = {}


def _build():
    nc = bacc.Bacc(None, target_bir_lowering=False)

    xq = nc.declare_dram_parameter("xq", [S, E], f32, isOutput=False)
    xk = nc.declare_dram_parameter("xk", [S, E], f32, isOutput=False)
    xv = nc.declare_dram_parameter("xv", [S, E], f32, isOutput=False)
    wq = nc.declare_dram_parameter("wq", [P, 8, HH], f32, isOutput=False)
    wk = nc.declare_dram_parameter("wk", [P, 8, HH], f32, isOutput=False)
    wv = nc.declare_dram_parameter("wv", [P, 8, HH], f32, isOutput=False)
    bq_col = nc.declare_dram_parameter("bq_col", [P, 4], f32, isOutput=False)
    bk_col = nc.declare_dram_parameter("bk_col", [P, 4], f32, isOutput=False)
    bv_row = nc.declare_dram_parameter("bv_row", [1, HH], f32, isOutput=False)
    wo = nc.declare_dram_parameter("wo", [P, 4, E], f32, isOutput=False)
    bo_col = nc.declare_dram_parameter("bo_col", [P, 8], f32, isOutput=False)
    yT = nc.declare_dram_parameter("yT", [E, S], f32, isOutput=True)

    from contextlib import ExitStack

    with tile.TileContext(nc) as tc, ExitStack() as stack:
        const = stack.enter_context(tc.tile_pool(name="const", bufs=1))
        qkv = stack.enter_context(tc.tile_pool(name="qkv", bufs=1))
        oup = stack.enter_context(tc.tile_pool(name="oup", bufs=1))

        ident = const.tile([P, P], f32)
        make_identity(nc, ident[:])

        onesf = const.tile([P, P], f32)
        nc.vector.memset(onesf[:], 1.0)
        # f32r constants (memset can't target f32r; cast-copy from fp32)
        ones_t = const.tile([P, P], f32r)       # rows 64 and 96 used as bcast lhsT
        nc.vector.tensor_copy(ones_t[64:65, :], onesf[64:65, :])
        nc.vector.tensor_copy(ones_t[96:97, :], onesf[96:97, :])
        onesk1 = const.tile([1, P], f32r)       # lhsT for v-bias matmul
        nc.vector.tensor_copy(onesk1[:], onesf[0:1, :])
        vones = const.tile([P, 16, 8], f32)     # ones column filler for vbuf
        nc.vector.memset(vones[:], 1.0)

        bqc = const.tile([P, 4], f32)
        bkc = const.tile([P, 4], f32)
        boc = const.tile([P, 8], f32)
        bvr = const.tile([1, HH], f32r)
        nc.sync.dma_start(out=bqc[:], in_=bq_col[:])
        nc.sync.dma_start(out=bkc[:], in_=bk_col[:])
        nc.sync.dma_start(out=boc[:], in_=bo_col[:])
        nc.gpsimd.dma_start(out=bvr[:], in_=bv_row[:])

        qT = qkv.tile([P, 4, S], f32r)          # [dq within tile, pair, sq]
        kT = qkv.tile([P, 4, S], f32r)
        vbuf = qkv.tile([P, 16, 8, D + 1], f32r)  # [sv, s-tile, head, d|1]
        ou = oup.tile([P, 4, S], f32r)          # normalized attn out, [hd, pair, sq]

        nc.vector.tensor_copy(vbuf[:, :, :, D], vones[:])

        # ---------------- Phase A: transposes + projections ----------------
        with tc.tile_pool(name="wp", bufs=1) as wp, \
             tc.tile_pool(name="xp", bufs=3) as xp, \
             tc.tile_pool(name="xtp", bufs=1) as xtp, \
             tc.tile_pool(name="ps_tr", bufs=3, space="PSUM") as ps_tr, \
             tc.tile_pool(name="ps_pj", bufs=3, space="PSUM") as ps_pj:
            for xdram, wdram, kind in ((xv, wv, "v"), (xk, wk, "k"), (xq, wq, "q")):
                w_t = wp.tile([P, 8, HH], f32r, tag="w")
                nc.gpsimd.dma_start(out=w_t[:], in_=wdram[:])
                for c in range(NCH):
                    xT_t = xtp.tile([P, 8, CH], f32r, tag="xT")
                    for i in range(4):
                        x_t = xp.tile([P, E], f32, tag="x")
                        r0 = (c * 4 + i) * P
                        nc.sync.dma_start(out=x_t[:], in_=xdram[r0:r0 + P, :])
                        for et in range(8):
                            pt = ps_tr.tile([P, P], f32, tag="tr")
                            nc.tensor.transpose(pt[:], x_t[:, et * P:(et + 1) * P], ident[:])
                            nc.vector.tensor_copy(xT_t[:, et, i * P:(i + 1) * P], pt[:])
                    if kind == "v":
                        for i in range(4):
                            pp = ps_pj.tile([P, 8, D], f32, tag="pj")
                            for et in range(8):
                                nc.tensor.matmul(pp[:], lhsT=xT_t[:, et, i * P:(i + 1) * P],
                                                 rhs=w_t[:, et, :], start=(et == 0), stop=False)
                            nc.tensor.matmul(pp[:], lhsT=onesk1[:], rhs=bvr[:],
                                             start=False, stop=True)
                            nc.vector.tensor_copy(vbuf[:, c * 4 + i, :, 0:D], pp[:])
                    else:
                        bcol = bqc if kind == "q" else bkc
                        dest = qT if kind == "q" else kT
                        for dt in range(4):
                            pp = ps_pj.tile([P, CH], f32, tag="pj")
                            for et in range(8):
                                nc.tensor.matmul(pp[:], lhsT=w_t[:, et, dt * P:(dt + 1) * P],
                                                 rhs=xT_t[:, et, :], start=(et == 0), stop=(et == 7))
                            nc.scalar.activation(dest[:, dt, c * CH:(c + 1) * CH], pp[:],
                                                 AF.Identity, bias=bcol[:, dt:dt + 1], scale=1.0)

        # ---------------- Phase B: attention + softmax + normalize ----------------
        with tc.tile_pool(name="ep", bufs=4) as ep, \
             tc.tile_pool(name="dnp", bufs=2) as dnp, \
             tc.tile_pool(name="rdp", bufs=2) as rdp, \
             tc.tile_pool(name="bcp", bufs=2) as bcp, \
             tc.tile_pool(name="ps_sc", bufs=2, space="PSUM") as ps_sc, \
             tc.tile_pool(name="ps_ac", bufs=2, space="PSUM") as ps_ac, \
             tc.tile_pool(name="ps_bc", bufs=2, space="PSUM") as ps_bc:
            for pr in range(4):
                hA, hB = 2 * pr, 2 * pr + 1
                for c in range(NCH):
                    cs = slice(c * CH, (c + 1) * CH)
                    psoA = ps_ac.tile([D + 1, CH], f32, tag="acc")
                    psoB = ps_ac.tile([D + 1, CH], f32, tag="acc")
                    for s in range(0, 16, 2):
                        for half, (pso, hh_) in enumerate(((psoA, hA), (psoB, hB))):
                            pb = slice(64 * half, 64 * half + 64)
                            psc = ps_sc.tile([P, 2, CH], f32, tag="sc")
                            for j in range(2):
                                st = s + j
                                nc.tensor.matmul(psc[:, j, :],
                                                 lhsT=kT[pb, pr, st * P:(st + 1) * P],
                                                 rhs=qT[pb, pr, cs],
                                                 start=True, stop=True)
                            ex = ep.tile([P, 2, CH], f32r, tag="expT")
                            nc.scalar.activation(ex[:], psc[:], AF.Exp, scale=0.125)
                            for j in range(2):
                                st = s + j
                                nc.tensor.matmul(pso[:], lhsT=vbuf[:, st, hh_, :],
                                                 rhs=ex[:, j, :],
                                                 start=(st == 0), stop=(st == 15),
                                                 skip_group_check=True)
                    # softmax denominators sit at psum row 64 (ones column)
                    den = dnp.tile([P, CH], f32, tag="den")
                    nc.vector.tensor_copy(den[64:65, :], psoA[64:65, :])
                    nc.vector.tensor_copy(den[96:97, :], psoB[64:65, :])
                    rden = rdp.tile([P, CH], f32r, tag="rden")
                    with nc.allow_low_precision(reason="softmax reciprocal in f32r"):
                        nc.vector.reciprocal(rden[64:65, :], den[64:65, :])
                        nc.vector.reciprocal(rden[96:97, :], den[96:97, :])
                    psbA = ps_bc.tile([64, CH], f32, tag="bc")
                    nc.tensor.matmul(psbA[:], lhsT=ones_t[64:65, 0:64],
                                     rhs=rden[64:65, :], start=True, stop=True,
                                     tile_position=(64, 0))
                    psbB = ps_bc.tile([64, CH], f32, tag="bc")
                    nc.tensor.matmul(psbB[:], lhsT=ones_t[96:97, 0:64],
                                     rhs=rden[96:97, :], start=True, stop=True,
                                     tile_position=(96, 0))
                    bcs = bcp.tile([P, CH], f32, tag="bcs")
                    nc.vector.tensor_copy(bcs[0:64, :], psbA[:])
                    nc.vector.tensor_copy(bcs[64:128, :], psbB[:])
                    with nc.allow_low_precision(reason="normalized attn out in f32r"):
                        nc.vector.tensor_mul(ou[0:64, pr, cs], psoA[0:64, :], bcs[0:64, :])
                        nc.vector.tensor_mul(ou[64:128, pr, cs], psoB[0:64, :], bcs[64:128, :])

        # ---------------- Phase C: output projection ----------------
        with tc.tile_pool(name="wop", bufs=1) as wop, \
             tc.tile_pool(name="otp", bufs=2) as otp, \
             tc.tile_pool(name="ps_ou", bufs=4, space="PSUM") as ps_ou:
            wo_t = wop.tile([P, 4, E], f32r)
            nc.gpsimd.dma_start(out=wo_t[:], in_=wo[:])
            for et in range(8):
                out_t = otp.tile([P, S], f32, tag="out")
                for c in range(NCH):
                    po = ps_ou.tile([P, CH], f32, tag="po")
                    for t in range(4):
                        nc.tensor.matmul(po[:], lhsT=wo_t[:, t, et * P:(et + 1) * P],
                                         rhs=ou[:, t, c * CH:(c + 1) * CH],
                                         start=(t == 0), stop=(t == 3))
                    nc.scalar.activation(out_t[:, c * CH:(c + 1) * CH], po[:],
                                         AF.Identity, bias=boc[:, et:et + 1], scale=1.0)
                nc.sync.dma_start(out=yT[et * P:(et + 1) * P, :], in_=out_t[:])

    nc.finalize()
    return nc


def _get_nc():
    if "nc" not in _cached:
        _cached["nc"] = _build()
    return _cached["nc"]


def _in_maps(query, key, value, Wq, bq, Wk, bk, Wv, bv, Wo, bo):
    query = np.asarray(query, np.float32)
    key = np.asarray(key, np.float32)
    value = np.asarray(value, np.float32)
    maps = []
    for c in range(N_CORES):
        b, hh = divmod(c, 2)
        sl = slice(hh * HH, (hh + 1) * HH)

        def wcols(W, nt):
            # [E, HH] -> [P, nt, HH-ish] with row tiles on partitions
            Ws = np.asarray(W, np.float32)[:, sl]
            return np.ascontiguousarray(Ws.reshape(nt, P, HH).transpose(1, 0, 2))

        wo_s = np.asarray(Wo, np.float32)[sl, :]                      # [512, E]
        wo_r = np.ascontiguousarray(wo_s.reshape(4, P, E).transpose(1, 0, 2))
        bo_c = (np.asarray(bo, np.float32).reshape(8, P).T if hh == 0
                else np.zeros((P, 8), np.float32))
        maps.append({
            "xq": np.ascontiguousarray(query[b]),
            "xk": np.ascontiguousarray(key[b]),
            "xv": np.ascontiguousarray(value[b]),
            "wq": wcols(Wq, 8),
            "wk": wcols(Wk, 8),
            "wv": wcols(Wv, 8),
            "bq_col": np.ascontiguousarray(np.asarray(bq, np.float32)[sl].reshape(4, P).T),
            "bk_col": np.ascontiguousarray(np.asarray(bk, np.float32)[sl].reshape(4, P).T),
            "bv_row": np.asarray(bv, np.float32)[sl].reshape(1, HH),
            "wo": wo_r,
            "bo_col": np.ascontiguousarray(bo_c),
        })
    return maps


def _assemble(results):
    outs = [results[c]["yT"] for c in range(N_CORES)]
    return np.stack([(outs[2 * b] + outs[2 * b + 1]).T for b in range(B)]).astype(np.float32)


def kernel(**inputs):
    nc = _get_nc()
    maps = _in_maps(**inputs)
    r = run_bass_kernel_spmd(nc, maps, list(range(N_CORES)))
    return _assemble(r.results)


def _ensure_ntff_hook():
    """Register the axon NTFF profiling hook (missing antenv.axon_hooks shim)."""
    import contextlib
    import ctypes
    import types

    try:
        from antenv.axon_hooks import get_axon_ntff_profile_hook
        if get_axon_ntff_profile_hook() is not None:
            return
    except ImportError:
        pass

    import antenv

    holder = {}
    mod = types.ModuleType("antenv.axon_hooks")
    mod.set_axon_ntff_profile_hook = lambda h: holder.__setitem__("h", h)
    mod.get_axon_ntff_profile_hook = lambda: holder.get("h")
    sys.modules["antenv.axon_hooks"] = mod
    antenv.axon_hooks = mod

    so_path = "/opt/axon/libaxon_pjrt.so"
    lib = ctypes.CDLL(so_path)
    if not hasattr(lib, "axon_start_nrt_profile"):
        return
    lib.axon_start_nrt_profile.argtypes = [ctypes.POINTER(ctypes.c_int64), ctypes.c_size_t]
    lib.axon_start_nrt_profile.restype = ctypes.c_int64
    lib.axon_stop_nrt_profile.argtypes = [ctypes.c_char_p]
    lib.axon_stop_nrt_profile.restype = ctypes.c_int64

    @contextlib.contextmanager
    def _hook(output_dir, device_ids):
        import jax

        jax.devices()
        if device_ids:
            ids = (ctypes.c_int64 * len(device_ids))(*device_ids)
            rc = lib.axon_start_nrt_profile(ids, len(device_ids))
        else:
            rc = lib.axon_start_nrt_profile(None, 0)
        if rc != 0:
            raise RuntimeError(f"axon_start_nrt_profile rc={rc}")
        try:
            yield
        finally:
            n = lib.axon_stop_nrt_profile(str(output_dir).encode())
            if n < 0:
                raise RuntimeError(f"axon_stop_nrt_profile rc={n}")

    mod.set_axon_ntff_profile_hook(_hook)


def kernel_traced(tmpdir=None, **inputs):
    """Like kernel() but with NTFF tracing; returns (output, exec_time_ns)."""
    _ensure_ntff_hook()
    import concourse.bass_utils as bu
    bu.upload_artifacts = lambda d: d  # no artifact bucket in this container
    nc = _get_nc()
    maps = _in_maps(**inputs)
    r = run_bass_kernel_spmd(nc, maps, list(range(N_CORES)), trace=True, tmpdir=tmpdir)
    return _assemble(r.results), r.exec_time_ns
